# revision 6
# baseline (speedup 1.0000x reference)
"""Trainium2 Bass kernel for nn_NeuralNetworkSimplified (binarized 4-layer MLP + BN).

Math
----
reference computes, per hidden layer l (gamma=1, beta=0, biases b_l arbitrary):
    z = sign(a) @ sign(W).T + sign(b)
    h = clip(batchnorm_train(z), -1, 1)
and the next layer only consumes sign(h).  Since batchnorm's rsqrt(var+eps) > 0
and gamma=1/beta=0, sign(h) = sign(z - mean_batch(z)); the sign(b) bias shifts
z and its mean equally, so it cancels.  The whole network therefore reduces to
exact integer arithmetic:
    S0 = sign(x)
    S_l = sign(S_{l-1} @ sign(W_l).T - colmean(S_{l-1} @ sign(W_l).T))
    out = S3 @ sign(W4).T + sign(b4)
All matmuls are +-1 x +-1 with exact fp32 accumulation; the column mean over the
full batch B=16384 is mean = (sign(W) @ colsum(S_prev)) / B, so data-parallel
cores only need to AllReduce the tiny colsum vector u = colsum(S_prev) per layer.

On device we represent sign values as +-0.5 (exact in fp8e4), which scales every
matmul output by 1/4 uniformly; the is_ge comparison against the scaled mean is
unchanged, and the final layer multiplies by 4 before adding sign(b4).

Sharding: batch 16384 -> 8 cores x 2048.  Activations live feature-major
(transposed) so the contraction dim sits on SBUF partitions; host passes x.T
shards and W_l.T (layout-only prep; all FLOPs incl. sign() run on device).
"""

import numpy as np
import ml_dtypes

B, D, H1, H2, H3, C = 16384, 3072, 2048, 2048, 1024, 512
NCORES = 8
BL = B // NCORES          # 2048 rows per core
NF = 512                  # batch free-dim chunk (psum tile width)
NCH = BL // NF            # 4 chunks
LAYERS = [(D, H1), (H1, H2), (H2, H3), (H3, C)]
UDIMS = [D, H1, H2]       # length of u vector feeding each hidden layer's mean

_CACHE = {}


def _build_module():
    import concourse.bass as bass
    import concourse.mybir as mybir
    import concourse.tile as tile
    from concourse import bacc

    mdt = mybir.dt
    FP8 = mdt.float8e4
    ALU = mybir.AluOpType

    nc = bacc.Bacc(
        "TRN2",
        target_bir_lowering=False,
        debug=False,
        num_devices=NCORES,
    )

    xT = nc.dram_tensor("xT", [D, BL], mdt.bfloat16, kind="ExternalInput").ap()
    wT = [
        nc.dram_tensor(f"w{i + 1}t", [K, H], mdt.bfloat16, kind="ExternalInput").ap()
        for i, (K, H) in enumerate(LAYERS)
    ]
    b4 = nc.dram_tensor("b4", [C, 1], mdt.float32, kind="ExternalInput").ap()
    outT = nc.dram_tensor("outT", [C, BL], mdt.float32, kind="ExternalOutput").ap()

    cc_in = [
        nc.dram_tensor(f"cc_in{i}", [128, U // 128], mdt.float32).ap()
        for i, U in enumerate(UDIMS)
    ]
    cc_out = [
        nc.dram_tensor(f"cc_out{i}", [128, U // 128], mdt.float32, addr_space="Shared").ap()
        for i, U in enumerate(UDIMS)
    ]

    with tile.TileContext(nc, num_cores=NCORES) as tc:
        with (
            tc.tile_pool(name="raw", bufs=3) as raw,       # staging bf16 tiles
            tc.tile_pool(name="sA", bufs=1) as sA,         # S0, S2
            tc.tile_pool(name="sB", bufs=1) as sB,         # S1, S3
            tc.tile_pool(name="wA", bufs=1) as wA,         # W1~, W3~
            tc.tile_pool(name="wB", bufs=1) as wB,         # W2~, W4~
            tc.tile_pool(name="stat", bufs=2) as stat,     # u/t/bias vectors
            tc.tile_pool(name="ou", bufs=4) as ou,         # output staging
            tc.tile_pool(name="pz", bufs=7, space="PSUM") as pz,
            tc.tile_pool(name="pt", bufs=1, space="PSUM") as pt,
        ):
            # ---- sign(b4) as +-1 per-partition vector, [128, C//128] ----
            b4_sb = raw.tile([128, C // 128], mdt.float32, tag="rawb")
            nc.sync.dma_start(b4_sb, b4.rearrange("(o p) q -> p (o q)", p=128))
            sb4 = stat.tile([128, C // 128], mdt.float32, tag="sb4")
            nc.any.tensor_scalar(
                out=sb4, in0=b4_sb, scalar1=0.0, scalar2=2.0,
                op0=ALU.is_ge, op1=ALU.mult,
            )
            nc.any.tensor_scalar_add(sb4, sb4, -1.0)

            # ---- binarize helper: bf16 dram (K,H) -> fp8 +-0.5 sbuf [128,K/128,H]
            def prep_w(pool, idx):
                K, H = LAYERS[idx]
                w8 = pool.tile([128, K // 128, H], FP8, tag="w")
                for k in range(K // 128):
                    rt = raw.tile([128, H], mdt.bfloat16, tag="raww")
                    nc.sync.dma_start(rt, wT[idx][k * 128:(k + 1) * 128, :])
                    nc.any.tensor_scalar(
                        out=w8[:, k, :], in0=rt, scalar1=0.0, scalar2=0.5,
                        op0=ALU.is_ge, op1=ALU.subtract,
                    )
                return w8

            # ---- S0 = sign(x)/2 (fp8), u0 = colsum(S0) ----
            S0 = sA.tile([128, D // 128, BL], FP8, tag="s")
            u0 = stat.tile([128, D // 128], mdt.float32, tag="u0")
            for k in range(D // 128):
                rt = raw.tile([128, BL], mdt.bfloat16, tag="rawx")
                nc.sync.dma_start(rt, xT[k * 128:(k + 1) * 128, :])
                nc.any.tensor_scalar(
                    out=S0[:, k, :], in0=rt, scalar1=0.0, scalar2=0.5,
                    op0=ALU.is_ge, op1=ALU.subtract,
                )
                nc.vector.tensor_reduce(
                    u0[:, k:k + 1], S0[:, k, :], mybir.AxisListType.X, ALU.add,
                )

            W8_1 = prep_w(wA, 0)
            W8_2 = prep_w(wB, 1)

            # ---- u AllReduce chain: sbuf u -> dram -> AR -> sbuf fp16 ----
            def allreduce_u(idx, u_sb):
                U = UDIMS[idx]
                nc.gpsimd.dma_start(cc_in[idx][:, :], u_sb[:, :])
                nc.gpsimd.collective_compute(
                    "AllReduce",
                    ALU.add,
                    replica_groups=[list(range(NCORES))],
                    ins=[cc_in[idx][:, :]],
                    outs=[cc_out[idx][:, :]],
                )
                ug = stat.tile([128, U // 128], mdt.float32, tag=f"ug{idx}")
                nc.gpsimd.dma_start(ug, cc_out[idx][:, :])
                u16 = stat.tile([128, U // 128], mdt.float16, tag=f"u16_{idx}")
                nc.any.tensor_copy(out=u16, in_=ug)
                return u16

            u16_0 = allreduce_u(0, u0)

            # ---- one layer: Z.T tiles = W~.T.T @ A, mean via t-pass, sign epilogue
            def layer(l, A8, W8, u16, S_out, u_out):
                K, H = LAYERS[l]
                KT, MT = K // 128, H // 128
                DR = mybir.MatmulPerfMode.DoubleRow
                for m in range(MT):
                    mc = slice(m * 128, (m + 1) * 128)
                    psums = [
                        pz.tile([128, NF], mdt.float32, tag="pz", name=f"pz{m}_{n}")
                        for n in range(NCH)
                    ]
                    for kp in range(KT // 2):
                        ks = slice(2 * kp, 2 * kp + 2)
                        st, sp = kp == 0, kp == KT // 2 - 1
                        wsl = W8[:, ks, mc]
                        for n in range(NCH):
                            nc.tensor.matmul(
                                psums[n], wsl, A8[:, ks, n * NF:(n + 1) * NF],
                                start=st, stop=sp, perf_mode=DR,
                            )
                    if l < 3:
                        # t-pass: colsum(Z)[m-block] = (W~ @ u_global)[m-block]
                        pt_m = pt.tile([128, 1], mdt.float32, tag="pt")
                        for k in range(KT):
                            nc.tensor.matmul(
                                pt_m, W8[:, k, mc], u16[:, k:k + 1],
                                start=(k == 0), stop=(k == KT - 1),
                            )
                        t_m = stat.tile([128, 1], mdt.float32, tag="tm")
                        nc.any.tensor_scalar_mul(t_m, pt_m, 1.0 / B)
                        for n in range(NCH):
                            nc.any.tensor_scalar(
                                out=S_out[:, m, n * NF:(n + 1) * NF],
                                in0=psums[n], scalar1=t_m, scalar2=0.5,
                                op0=ALU.is_ge, op1=ALU.subtract,
                            )
                        if u_out is not None:
                            nc.vector.tensor_reduce(
                                u_out[:, m:m + 1], S_out[:, m, :],
                                mybir.AxisListType.X, ALU.add,
                            )
                    else:
                        for n in range(NCH):
                            ot = ou.tile([128, NF], mdt.float32, tag="ot")
                            nc.any.tensor_scalar(
                                out=ot, in0=psums[n],
                                scalar1=4.0, scalar2=sb4[:, m:m + 1],
                                op0=ALU.mult, op1=ALU.add,
                            )
                            nc.sync.dma_start(
                                outT[mc, n * NF:(n + 1) * NF], ot
                            )

            # layer 1
            S1 = sB.tile([128, H1 // 128, BL], FP8, tag="s")
            u1 = stat.tile([128, H1 // 128], mdt.float32, tag="u1")
            layer(0, S0, W8_1, u16_0, S1, u1)
            u16_1 = allreduce_u(1, u1)

            # layer 2 (W3 prep overlaps)
            W8_3 = prep_w(wA, 2)
            S2 = sA.tile([128, H2 // 128, BL], FP8, tag="s")
            u2 = stat.tile([128, H2 // 128], mdt.float32, tag="u2")
            layer(1, S1, W8_2, u16_1, S2, u2)
            u16_2 = allreduce_u(2, u2)

            # layer 3 (W4 prep overlaps)
            W8_4 = prep_w(wB, 3)
            S3 = sB.tile([128, H3 // 128, BL], FP8, tag="s")
            layer(2, S2, W8_3, u16_2, S3, None)

            # layer 4 (no BN)
            layer(3, S3, W8_4, None, None, None)

    nc.compile()
    return nc


def _get_module():
    if "nc" not in _CACHE:
        _CACHE["nc"] = _build_module()
    return _CACHE["nc"]


def _reference_fallback(x, W1, b1, g1, be1, W2, b2, g2, be2, W3, b3, g3, be3, W4, b4):
    """Exact numpy clone of the reference for non-trivial gamma/beta inputs."""
    EPS = 1e-5

    def binarize(v):
        return np.where(v >= 0, 1.0, -1.0).astype(np.float32)

    def bin_linear(a, W, b):
        return binarize(a) @ binarize(W).T + binarize(b)

    def bn(z, g, be):
        m = z.mean(axis=0)
        v = z.var(axis=0)
        return (z - m) / np.sqrt(v + EPS) * g + be

    h = np.clip(bn(bin_linear(x, W1, b1), g1, be1), -1.0, 1.0)
    h = np.clip(bn(bin_linear(h, W2, b2), g2, be2), -1.0, 1.0)
    h = np.clip(bn(bin_linear(h, W3, b3), g3, be3), -1.0, 1.0)
    return bin_linear(h, W4, b4).astype(np.float32)


def make_in_maps(inputs):
    bf16 = ml_dtypes.bfloat16
    x = inputs["x"]
    common = {
        "w1t": np.ascontiguousarray(np.asarray(inputs["W1"]).T).astype(bf16),
        "w2t": np.ascontiguousarray(np.asarray(inputs["W2"]).T).astype(bf16),
        "w3t": np.ascontiguousarray(np.asarray(inputs["W3"]).T).astype(bf16),
        "w4t": np.ascontiguousarray(np.asarray(inputs["W4"]).T).astype(bf16),
        "b4": np.asarray(inputs["b4"], dtype=np.float32).reshape(C, 1),
    }
    in_maps = []
    for c in range(NCORES):
        m = dict(common)
        m["xT"] = np.ascontiguousarray(
            np.asarray(x[c * BL:(c + 1) * BL, :]).T
        ).astype(bf16)
        in_maps.append(m)
    return in_maps


def gather_output(results):
    out = np.empty((B, C), dtype=np.float32)
    for c in range(NCORES):
        out[c * BL:(c + 1) * BL, :] = results[c]["outT"].T
    return out


def kernel(**inputs):
    # BN gamma/beta must be trivial for the sign-reduction; spec fills guarantee
    # this (g=ones, be=zeros).  Anything else falls back to exact host compute.
    for gk, bek in (("g1", "be1"), ("g2", "be2"), ("g3", "be3")):
        if not (np.all(np.asarray(inputs[gk]) == 1.0)
                and np.all(np.asarray(inputs[bek]) == 0.0)):
            return _reference_fallback(**{
                k: np.asarray(v, dtype=np.float32) for k, v in inputs.items()
            })

    from concourse.bass_utils import run_bass_kernel_spmd

    nc = _get_module()
    in_maps = make_in_maps(inputs)
    res = run_bass_kernel_spmd(nc, in_maps, list(range(NCORES)))
    return gather_output(res.results)


if __name__ == "__main__":
    nc = _get_module()
    print("module built OK")


# revision 11
# speedup vs baseline: 889.1537x; 889.1537x over previous
"""Trainium2 Bass kernel for nn_NeuralNetworkSimplified (binarized 4-layer MLP + BN).

Math
----
reference computes, per hidden layer l (gamma=1, beta=0, biases b_l arbitrary):
    z = sign(a) @ sign(W).T + sign(b)
    h = clip(batchnorm_train(z), -1, 1)
and the next layer only consumes sign(h).  Since batchnorm's rsqrt(var+eps) > 0
and gamma=1/beta=0, sign(h) = sign(z - mean_batch(z)); the sign(b) bias shifts
z and its mean equally, so it cancels.  The whole network therefore reduces to
exact integer arithmetic:
    S0 = sign(x)
    S_l = sign(S_{l-1} @ sign(W_l).T - colmean(S_{l-1} @ sign(W_l).T))
    out = S3 @ sign(W4).T + sign(b4)
All matmuls are +-1 x +-1 with exact fp32 accumulation; the column mean over the
full batch B=16384 is mean = (sign(W) @ colsum(S_prev)) / B, so data-parallel
cores only need to AllReduce the tiny colsum vector u = colsum(S_prev) per layer.

On device we represent sign values as +-0.5 (exact in fp8e4), which scales every
matmul output by 1/4 uniformly; the is_ge comparison against the scaled mean is
unchanged, and the final layer multiplies by 4 before adding sign(b4).

Sharding: batch 16384 -> 8 cores x 2048.  Activations live feature-major
(transposed) so the contraction dim sits on SBUF partitions; host passes x.T
shards and W_l.T (layout-only prep; all FLOPs incl. sign() run on device).

Layout: all sign tensors are lists of DoubleRow pair tiles [128, 2, free] so
Tile's dependency tracking overlaps binarize-prep with the first layer's
matmuls at pair granularity.
"""

import numpy as np
import ml_dtypes

B, D, H1, H2, H3, C = 16384, 3072, 2048, 2048, 1024, 512
NCORES = 8
BL = B // NCORES          # 2048 rows per core
NF = 512                  # batch free-dim chunk (psum tile width)
NCH = BL // NF            # 4 chunks
LAYERS = [(D, H1), (H1, H2), (H2, H3), (H3, C)]
UDIMS = [D, H1, H2]       # length of u vector feeding each hidden layer's mean

_CACHE = {}


def _build_module():
    import concourse.bass as bass
    import concourse.mybir as mybir
    import concourse.tile as tile
    from concourse import bacc

    mdt = mybir.dt
    FP8 = mdt.float8e4
    ALU = mybir.AluOpType

    nc = bacc.Bacc(
        "TRN2",
        target_bir_lowering=False,
        debug=False,
        num_devices=NCORES,
    )

    xT = nc.dram_tensor("xT", [D, BL], mdt.bfloat16, kind="ExternalInput").ap()
    wT = [
        nc.dram_tensor(f"w{i + 1}t", [K, H], mdt.bfloat16, kind="ExternalInput").ap()
        for i, (K, H) in enumerate(LAYERS)
    ]
    b4 = nc.dram_tensor("b4", [C, 1], mdt.float32, kind="ExternalInput").ap()
    outT = nc.dram_tensor("outT", [C, BL], mdt.float32, kind="ExternalOutput").ap()

    cc_in = [
        nc.dram_tensor(f"cc_in{i}", [128, U // 128], mdt.float32).ap()
        for i, U in enumerate(UDIMS)
    ]
    cc_out = [
        nc.dram_tensor(f"cc_out{i}", [128, U // 128], mdt.float32, addr_space="Shared").ap()
        for i, U in enumerate(UDIMS)
    ]

    with tile.TileContext(nc, num_cores=NCORES) as tc:
        with (
            tc.tile_pool(name="raw", bufs=2) as raw,       # staging bf16 pair tiles
            tc.tile_pool(name="sA", bufs=12) as sA,        # S0, S2 pair tiles
            tc.tile_pool(name="sB", bufs=8) as sB,         # S1, S3 pair tiles
            tc.tile_pool(name="wA", bufs=12) as wA,        # W1~, W3~ pair tiles
            tc.tile_pool(name="wB", bufs=8) as wB,         # W2~, W4~ pair tiles
            tc.tile_pool(name="stat", bufs=2) as stat,     # u/t/bias vectors
            tc.tile_pool(name="ou", bufs=4) as ou,         # output staging
            tc.tile_pool(name="pz", bufs=7, space="PSUM") as pz,
            tc.tile_pool(name="pt", bufs=1, space="PSUM") as pt,
        ):
            # ---- PE warm-up: ~5us of dummy matmuls so HAM unthrottles while
            # the binarize prologue runs on DMA/DVE.
            warm = stat.tile([128, 128], FP8, tag="warm")
            nc.vector.memset(warm, 0.5)
            wps = pz.tile([128, 128], mdt.float32, tag="pz", name="warmps")
            for i in range(24):
                nc.tensor.matmul(wps, warm, warm, start=True, stop=True)

            # ---- sign(b4) as +-1 per-partition vector, [128, C//128] ----
            b4_sb = raw.tile([128, C // 128], mdt.float32, tag="rawb")
            nc.sync.dma_start(b4_sb, b4.rearrange("(o p) q -> p (o q)", p=128))
            sb4 = stat.tile([128, C // 128], mdt.float32, tag="sb4")
            nc.any.tensor_scalar(
                out=sb4, in0=b4_sb, scalar1=0.0, scalar2=2.0,
                op0=ALU.is_ge, op1=ALU.mult,
            )
            nc.any.tensor_scalar_add(sb4, sb4, -1.0)

            # ---- binarize helper: bf16 dram (K,H) -> fp8 +-0.5 pair tiles ----
            def prep_w(pool, idx):
                K, H = LAYERS[idx]
                pairs = []
                for kp in range(K // 256):
                    rt = raw.tile([128, 2, H], mdt.bfloat16, tag="raww",
                                  name=f"rw{idx}_{kp}")
                    w8 = pool.tile([128, 2, H], FP8, tag="w", name=f"w{idx}_{kp}")
                    for h in range(2):
                        nc.sync.dma_start(
                            rt[:, h, :],
                            wT[idx][(2 * kp + h) * 128:(2 * kp + h + 1) * 128, :],
                        )
                        nc.any.tensor_scalar(
                            out=w8[:, h, :], in0=rt[:, h, :], scalar1=0.0,
                            scalar2=0.5, op0=ALU.is_ge, op1=ALU.subtract,
                        )
                    pairs.append(w8)
                return pairs

            # ---- S0 = sign(x)/2 (fp8 pair tiles), u0 = colsum(S0) fused ----
            S0 = []
            u0 = stat.tile([128, D // 128], mdt.float32, tag="u0")
            for kp in range(D // 256):
                rt = raw.tile([128, 2, BL], mdt.bfloat16, tag="rawx",
                              name=f"rx{kp}")
                s8 = sA.tile([128, 2, BL], FP8, tag="s", name=f"s0_{kp}")
                for h in range(2):
                    nc.sync.dma_start(
                        rt[:, h, :],
                        xT[(2 * kp + h) * 128:(2 * kp + h + 1) * 128, :],
                    )
                    nc.any.tensor_scalar(
                        out=s8[:, h, :], in0=rt[:, h, :], scalar1=0.0,
                        scalar2=0.5, op0=ALU.is_ge, op1=ALU.subtract,
                    )
                    nc.vector.tensor_reduce(
                        u0[:, 2 * kp + h:2 * kp + h + 1], s8[:, h, :],
                        mybir.AxisListType.X, ALU.add,
                    )
                S0.append(s8)

            W8_1 = prep_w(wA, 0)
            W8_2 = prep_w(wB, 1)

            # ---- u AllReduce chain: sbuf u -> dram -> AR -> sbuf fp16 ----
            def allreduce_u(idx, u_sb):
                U = UDIMS[idx]
                nc.gpsimd.dma_start(cc_in[idx][:, :], u_sb[:, :])
                nc.gpsimd.collective_compute(
                    "AllReduce",
                    ALU.add,
                    replica_groups=[list(range(NCORES))],
                    ins=[cc_in[idx][:, :]],
                    outs=[cc_out[idx][:, :]],
                )
                ug = stat.tile([128, U // 128], mdt.float32, tag=f"ug{idx}")
                nc.gpsimd.dma_start(ug, cc_out[idx][:, :])
                u16 = stat.tile([128, U // 128], mdt.float16, tag=f"u16_{idx}")
                nc.any.tensor_copy(out=u16, in_=ug)
                return u16

            u16_0 = allreduce_u(0, u0)

            # ---- one layer ----
            def layer(l, A8, W8, u16, S_out, u_out):
                K, H = LAYERS[l]
                KT, MT = K // 128, H // 128
                DR = mybir.MatmulPerfMode.DoubleRow
                for m in range(MT):
                    mc = slice(m * 128, (m + 1) * 128)
                    psums = [
                        pz.tile([128, NF], mdt.float32, tag="pz", name=f"pz{m}_{n}")
                        for n in range(NCH)
                    ]
                    for kp in range(KT // 2):
                        st, sp = kp == 0, kp == KT // 2 - 1
                        wsl = W8[kp][:, :, mc]
                        for n in range(NCH):
                            nc.tensor.matmul(
                                psums[n], wsl, A8[kp][:, :, n * NF:(n + 1) * NF],
                                start=st, stop=sp, perf_mode=DR,
                            )
                    if l < 3:
                        # t-pass: colsum(Z)[m-block] = (W~ @ u_global)[m-block]
                        pt_m = pt.tile([128, 1], mdt.float32, tag="pt",
                                       name=f"pt{m}")
                        for k in range(KT):
                            nc.tensor.matmul(
                                pt_m, W8[k // 2][:, k % 2, mc], u16[:, k:k + 1],
                                start=(k == 0), stop=(k == KT - 1),
                            )
                        t_m = stat.tile([128, 1], mdt.float32, tag="tm",
                                        name=f"tm{m}")
                        nc.any.tensor_scalar_mul(t_m, pt_m, 1.0 / B)
                        for n in range(NCH):
                            nc.any.tensor_scalar(
                                out=S_out[m // 2][:, m % 2, n * NF:(n + 1) * NF],
                                in0=psums[n], scalar1=t_m, scalar2=0.5,
                                op0=ALU.is_ge, op1=ALU.subtract,
                            )
                        if u_out is not None:
                            nc.vector.tensor_reduce(
                                u_out[:, m:m + 1],
                                S_out[m // 2][:, m % 2, :],
                                mybir.AxisListType.X, ALU.add,
                            )
                    else:
                        for n in range(NCH):
                            ot = ou.tile([128, NF], mdt.float32, tag="ot",
                                         name=f"ot{m}_{n}")
                            nc.any.tensor_scalar(
                                out=ot, in0=psums[n],
                                scalar1=4.0, scalar2=sb4[:, m:m + 1],
                                op0=ALU.mult, op1=ALU.add,
                            )
                            nc.sync.dma_start(
                                outT[mc, n * NF:(n + 1) * NF], ot
                            )

            def alloc_s(pool, H, nm):
                return [
                    pool.tile([128, 2, BL], FP8, tag="s", name=f"{nm}_{i}")
                    for i in range(H // 256)
                ]

            # layer 1
            S1 = alloc_s(sB, H1, "s1")
            u1 = stat.tile([128, H1 // 128], mdt.float32, tag="u1")
            layer(0, S0, W8_1, u16_0, S1, u1)
            u16_1 = allreduce_u(1, u1)

            # layer 2 (W3 prep overlaps)
            W8_3 = prep_w(wA, 2)
            S2 = alloc_s(sA, H2, "s2")
            u2 = stat.tile([128, H2 // 128], mdt.float32, tag="u2")
            layer(1, S1, W8_2, u16_1, S2, u2)
            u16_2 = allreduce_u(2, u2)

            # layer 3 (W4 prep overlaps)
            W8_4 = prep_w(wB, 3)
            S3 = alloc_s(sB, H3, "s3")
            layer(2, S2, W8_3, u16_2, S3, None)

            # layer 4 (no BN)
            layer(3, S3, W8_4, None, None, None)

    nc.compile()
    return nc


def _get_module():
    if "nc" not in _CACHE:
        _CACHE["nc"] = _build_module()
    return _CACHE["nc"]


def _reference_fallback(x, W1, b1, g1, be1, W2, b2, g2, be2, W3, b3, g3, be3, W4, b4):
    """Exact numpy clone of the reference for non-trivial gamma/beta inputs."""
    EPS = 1e-5

    def binarize(v):
        return np.where(v >= 0, 1.0, -1.0).astype(np.float32)

    def bin_linear(a, W, b):
        return binarize(a) @ binarize(W).T + binarize(b)

    def bn(z, g, be):
        m = z.mean(axis=0)
        v = z.var(axis=0)
        return (z - m) / np.sqrt(v + EPS) * g + be

    h = np.clip(bn(bin_linear(x, W1, b1), g1, be1), -1.0, 1.0)
    h = np.clip(bn(bin_linear(h, W2, b2), g2, be2), -1.0, 1.0)
    h = np.clip(bn(bin_linear(h, W3, b3), g3, be3), -1.0, 1.0)
    return bin_linear(h, W4, b4).astype(np.float32)


def make_in_maps(inputs):
    bf16 = ml_dtypes.bfloat16
    x = inputs["x"]
    common = {
        "w1t": np.ascontiguousarray(np.asarray(inputs["W1"]).T).astype(bf16),
        "w2t": np.ascontiguousarray(np.asarray(inputs["W2"]).T).astype(bf16),
        "w3t": np.ascontiguousarray(np.asarray(inputs["W3"]).T).astype(bf16),
        "w4t": np.ascontiguousarray(np.asarray(inputs["W4"]).T).astype(bf16),
        "b4": np.asarray(inputs["b4"], dtype=np.float32).reshape(C, 1),
    }
    in_maps = []
    for c in range(NCORES):
        m = dict(common)
        m["xT"] = np.ascontiguousarray(
            np.asarray(x[c * BL:(c + 1) * BL, :]).T
        ).astype(bf16)
        in_maps.append(m)
    return in_maps


def gather_output(results):
    out = np.empty((B, C), dtype=np.float32)
    for c in range(NCORES):
        out[c * BL:(c + 1) * BL, :] = results[c]["outT"].T
    return out


def kernel(**inputs):
    # BN gamma/beta must be trivial for the sign-reduction; spec fills guarantee
    # this (g=ones, be=zeros).  Anything else falls back to exact host compute.
    for gk, bek in (("g1", "be1"), ("g2", "be2"), ("g3", "be3")):
        if not (np.all(np.asarray(inputs[gk]) == 1.0)
                and np.all(np.asarray(inputs[bek]) == 0.0)):
            return _reference_fallback(**{
                k: np.asarray(v, dtype=np.float32) for k, v in inputs.items()
            })

    from concourse.bass_utils import run_bass_kernel_spmd

    nc = _get_module()
    in_maps = make_in_maps(inputs)
    res = run_bass_kernel_spmd(nc, in_maps, list(range(NCORES)))
    return gather_output(res.results)


if __name__ == "__main__":
    nc = _get_module()
    print("module built OK")
